# revision 1
# baseline (speedup 1.0000x reference)
"""AttentionMV pooling kernel for Trainium2 (Bass/Tile), 8-core data-parallel.

Computes, for full inputs x:(64,2048,1024) c:(64,1024) W:(1024,1) b:(2048,1)
U:(1024,2048):
    et = c @ U + (x @ W)[..., 0] + b[:, 0]        # (B, T)
    at = softmax(et, axis=-1)
    out = einsum('bt,bte->be', at, x)             # (B, E)

Sharding: data-parallel over batch B across the 8 NeuronCores (8 batches per
core); W/b/U replicated. No collectives; the host concatenates per-core
outputs.

Per-core dataflow (x read from HBM exactly once):
  1. ct[t, b] = sum_e U[e,t] c[b,e] + bias[t]  on PE (c transposed on-chip
     via identity matmuls), stored [t%128, t//128, b].
  2. Per x tile [128t x 1024e]: one fused DVE tensor_tensor_reduce computes
     et tile = sum_e (x * W) + ct  (x stays resident in SBUF).
  3. exp(et - 10) on ACT with accum_out row sums; denominator via a
     ones-matmul partition reduction; reciprocal folded into the final scale.
  4. out[b] = sum_t at[t] x[t,:] as accumulating PE matmuls with at as the
     [128,1] stationary operand, re-using the resident x tiles.
"""

import os

import numpy as np

import concourse.bass as bass
import concourse.mybir as mybir
import concourse.tile as tile
from concourse import bacc
from concourse.masks import make_identity

B, T, E = 64, 2048, 1024
NCORES = 8
BL = B // NCORES  # local batches per core
P = 128
NT = T // P  # 16 t-chunks of 128
NSUB = 2  # t-chunks per DMA tile
KTILES = NT // NSUB  # 8 x-tiles per batch
NE = E // P  # 8 e-chunks
F32 = mybir.dt.float32
F32R = mybir.dt.float32r
SHIFT = 10.0  # softmax exp shift; cancels exactly in the normalization

_CACHE = {}
LAST_RESULTS = None  # BassKernelResults of the most recent run (for test harness)


def build_bass():
    nc = bacc.Bacc(None, target_bir_lowering=False)

    x = nc.dram_tensor("x", [BL, T, E], F32, kind="ExternalInput")
    c = nc.dram_tensor("c", [BL, E], F32, kind="ExternalInput")
    W = nc.dram_tensor("W", [E, 1], F32, kind="ExternalInput")
    bias = nc.dram_tensor("b", [T, 1], F32, kind="ExternalInput")
    U = nc.dram_tensor("U", [E, T], F32, kind="ExternalInput")
    out = nc.dram_tensor("out", [BL, E], F32, kind="ExternalOutput")
    den_out = nc.dram_tensor("den", [1, BL], F32, kind="ExternalOutput")

    with tile.TileContext(nc) as tc:
        with (
            tc.tile_pool(name="big", bufs=21) as big,
            tc.tile_pool(name="singles", bufs=1) as singles,
            tc.tile_pool(name="pb", bufs=3) as pb,
            tc.tile_pool(name="psum", bufs=2, space="PSUM") as psum,
        ):
            # ---------------- constants / small inputs ----------------
            w_bc = singles.tile([P, E], F32)  # W broadcast to all partitions
            wap = W[:, 0:1]
            nc.gpsimd.dma_start(
                out=w_bc,
                in_=bass.AP(tensor=wap.tensor, offset=wap.offset, ap=[[0, P], [1, E]]),
            )

            # bias[t] laid out [t%128, t//128] so it can be an ACT per-partition bias
            bias_pt = singles.tile([P, NT], F32)
            bap = bias[:, 0:1]
            nc.gpsimd.dma_start(
                out=bias_pt,
                in_=bass.AP(tensor=bap.tensor, offset=bap.offset, ap=[[1, P], [P, NT]]),
            )

            c_sb = singles.tile([BL, E], F32)
            nc.sync.dma_start(out=c_sb, in_=c[:, :])

            id8 = singles.tile([BL, BL], F32)
            make_identity(nc, id8)

            ones_p1 = singles.tile([P, 1], F32)
            nc.vector.memset(ones_p1, 1.0)
            # f32r matmuls reject a free dim of 1; use two ones columns
            ones_r = singles.tile([P, 2], F32R)
            nc.scalar.copy(out=ones_r[:, 0:1], in_=ones_p1)
            nc.scalar.copy(out=ones_r[:, 1:2], in_=ones_p1)
            # fold the fixed softmax shift into the bias that lands in ct_all
            shift_sb = singles.tile([P, 1], F32)
            nc.vector.memset(shift_sb, -SHIFT)
            nc.scalar.activation(
                out=bias_pt,
                in_=bias_pt,
                func=mybir.ActivationFunctionType.Identity,
                bias=shift_sb,
                scale=1.0,
            )

            # ---------------- transpose c: cT[e, j, b] ----------------
            cT = singles.tile([P, NE, BL], F32R)
            for j in range(NE):
                tp = psum.tile([P, BL], F32, tag="tr", bufs=1)
                nc.tensor.matmul(
                    tp, lhsT=c_sb[:, j * P : (j + 1) * P], rhs=id8, start=True, stop=True
                )
                nc.scalar.copy(out=cT[:, j, :], in_=tp)

            # Mostly U first (every batch's exp waits on ct), but slip the
            # first two x tiles in early so the DVE pipeline primes at once.
            xr = x[:, :, :].rearrange("b (k n p) e -> b k p n e", n=NSUB, p=P)
            ur = U[:, :].rearrange("(j p) t -> j p t", p=P)
            xts0 = []
            for k in range(2):
                xt = big.tile([P, NSUB, E], F32R, tag="big", name=f"x0_{k}")
                nc.sync.dma_start(out=xt, in_=xr[0, k].bitcast(F32R))
                xts0.append(xt)
            u_tiles = []
            for j in range(NE):
                ut = big.tile([P, T], F32R, tag="big", name=f"u{j}")
                nc.sync.dma_start(out=ut, in_=ur[j].bitcast(F32R))
                u_tiles.append(ut)
            for k in range(2, KTILES):
                xt = big.tile([P, NSUB, E], F32R, tag="big", name=f"x0_{k}")
                nc.sync.dma_start(out=xt, in_=xr[0, k].bitcast(F32R))
                xts0.append(xt)

            # ---------------- ct = U.T @ cT + bias ----------------
            # One single-bank PSUM tile holds all 16 t-chunk accumulators as
            # element-disjoint regions; U tiles are consumed as they arrive
            # (j outer), freeing their pool slots immediately.
            # start=True zeroes the whole 2KB zero-region (= this bank), so
            # only the very first matmul starts; everything else accumulates.
            ct_ps = psum.tile([P, NT, BL], F32, tag="ctacc", bufs=1)
            for j in range(NE):
                for i in range(NT):
                    nc.tensor.matmul(
                        ct_ps[:, i, :],
                        lhsT=u_tiles[j][:, i * P : (i + 1) * P],
                        rhs=cT[:, j, :],
                        start=(j == 0 and i == 0),
                        stop=(j == NE - 1 and i == NT - 1),
                    )

            # ct_all[p, i, b] = ct[i*128 + p, b] + bias[i*128 + p]
            ct_all = singles.tile([P, NT, BL], F32)
            for i in range(NT):
                nc.scalar.activation(
                    out=ct_all[:, i, :],
                    in_=ct_ps[:, i, :],
                    func=mybir.ActivationFunctionType.Identity,
                    bias=bias_pt[:, i : i + 1],
                    scale=1.0,
                )

            # ---------------- main loop over local batches ----------------
            # full-size scratch for the fused multiply's elementwise output
            # (tensor_tensor_reduce hangs this HW's DVE ucode; use
            # scalar_tensor_tensor + accum_out instead, then add ct after)
            scratch = singles.tile([P, E], F32)
            den_all = singles.tile([1, BL], F32)

            for bi in range(BL):
                if bi == 0:
                    xts = xts0
                else:
                    xts = []
                    for k in range(KTILES):
                        xt = big.tile([P, NSUB, E], F32R, tag="big", name=f"x{bi}_{k}")
                        nc.sync.dma_start(out=xt, in_=xr[bi, k].bitcast(F32R))
                        xts.append(xt)

                # Tile-granular pipeline: the softmax shift is a constant (not
                # the row max), so each t-chunk's exp contribution is
                # independent — no per-batch barrier anywhere.
                dps = psum.tile([1, 2], F32, tag="den")
                ops = psum.tile([1, E], F32, tag="out")
                for k in range(KTILES):
                    for n in range(NSUB):
                        i = k * NSUB + n
                        # et_i = sum_e x[t, e] * W[e]
                        et_i = pb.tile([P, 1], F32, tag="et", bufs=6, name=f"et{bi}_{i}")
                        nc.vector.scalar_tensor_tensor(
                            out=scratch,
                            in0=xts[k][:, n, :].bitcast(F32),
                            scalar=0.0,
                            in1=w_bc,
                            op0=mybir.AluOpType.add,
                            op1=mybir.AluOpType.mult,
                            accum_out=et_i,
                        )
                        # at_i = exp(et_i + ct + bias - SHIFT)
                        ev_i = pb.tile(
                            [P, 1], F32R, tag="ev", bufs=6, name=f"ev{bi}_{i}"
                        )
                        nc.scalar.activation(
                            out=ev_i,
                            in_=et_i,
                            func=mybir.ActivationFunctionType.Exp,
                            bias=ct_all[:, i, bi : bi + 1],
                            scale=1.0,
                        )
                        # denominator contribution + weighted sum of x rows
                        nc.tensor.matmul(
                            dps,
                            lhsT=ev_i,
                            rhs=ones_r,
                            start=(i == 0),
                            stop=(i == NT - 1),
                        )
                        for h in range(2):
                            nc.tensor.matmul(
                                ops[:, h * 512 : (h + 1) * 512],
                                lhsT=ev_i,
                                rhs=xts[k][:, n, h * 512 : (h + 1) * 512],
                                start=(i == 0),
                                stop=(i == NT - 1),
                            )

                nc.scalar.copy(out=den_all[:, bi : bi + 1], in_=dps[:, 0:1])
                out_sb = pb.tile([1, E], F32, tag="out_sb")
                nc.scalar.copy(out=out_sb, in_=ops)
                # gpsimd (SWDGE) queue: a sync-queue store here would block
                # the next batches' x-tile loads behind it (SP issues in order)
                nc.gpsimd.dma_start(out=out[bi : bi + 1, :], in_=out_sb)

            nc.gpsimd.dma_start(out=den_out[:, :], in_=den_all)

    nc.compile()
    return nc


def _get_exec():
    """Build the Bass program once and return (nc, in_names, out_names,
    out_avals, jitted _body). The multi-device shard_map path hangs through
    the axon tunnel, so we run 8 independent single-device executions
    instead (the kernel has no collectives)."""
    if "exec" in _CACHE:
        return _CACHE["exec"]

    import jax
    from concourse import bass2jax, mybir as _mybir

    bass2jax.install_neuronx_cc_hook()
    nc = build_bass()

    in_names, out_names, out_avals, zero_shapes = [], [], [], []
    for alloc in nc.m.functions[0].allocations:
        if not isinstance(alloc, _mybir.MemoryLocationSet):
            continue
        name = alloc.memorylocations[0].name
        if alloc.kind == "ExternalInput":
            in_names.append(name)
        elif alloc.kind == "ExternalOutput":
            out_names.append(name)
            shape = tuple(alloc.tensor_shape)
            dtype = _mybir.dt.np(alloc.dtype)
            out_avals.append(jax.core.ShapedArray(shape, dtype))
            zero_shapes.append((shape, dtype))
    n_params = len(in_names)
    all_names = in_names + out_names
    donate = tuple(range(n_params, n_params + len(out_names)))

    def _body(*args):
        outs = bass2jax._bass_exec_p.bind(
            *args,
            out_avals=tuple(out_avals),
            in_names=tuple(all_names),
            out_names=tuple(out_names),
            lowering_input_output_aliases=(),
            sim_require_finite=True,
            sim_require_nnan=True,
            nc=nc,
        )
        return tuple(outs)

    jitted = jax.jit(_body, donate_argnums=donate, keep_unused=True)
    _CACHE["exec"] = (nc, in_names, out_names, zero_shapes, jitted)
    return _CACHE["exec"]


_VERBOSE = os.environ.get("BASS_KERNEL_VERBOSE", "0") == "1"


def _log(msg):
    if _VERBOSE:
        import time

        print(f"[kernel {time.strftime('%H:%M:%S')}] {msg}", flush=True)


def kernel(x, c, W, b, U, trace=False, sequential=None):
    import jax

    nc, in_names, out_names, zero_shapes, jitted = _get_exec()

    x = np.ascontiguousarray(x, dtype=np.float32)
    c = np.ascontiguousarray(c, dtype=np.float32)
    W = np.ascontiguousarray(W, dtype=np.float32)
    b = np.ascontiguousarray(b, dtype=np.float32)
    U = np.ascontiguousarray(U, dtype=np.float32)

    if sequential is None:
        sequential = os.environ.get("BASS_KERNEL_SEQUENTIAL", "0") == "1"

    devices = jax.devices()[:NCORES]

    def _dispatch(k, dev):
        per_core = {
            "x": x[k * BL : (k + 1) * BL],
            "c": c[k * BL : (k + 1) * BL],
            "W": W,
            "b": b,
            "U": U,
        }
        if nc.partition_id_tensor is not None:
            pid = nc.partition_id_tensor
            per_core[pid.name] = np.full(pid.shape, k, dtype=mybir.dt.np(pid.dtype))
        _log(f"core {k}: device_put")
        args = [
            jax.device_put(np.ascontiguousarray(per_core[n]), dev) for n in in_names
        ]
        args += [
            jax.device_put(np.zeros(shape, dtype), dev) for shape, dtype in zero_shapes
        ]
        _log(f"core {k}: launch")
        return jitted(*args)

    def _final(res):
        # normalize on the host: out / den (softmax denominator per batch)
        den = res["den"][0].astype(np.float64)
        return (res["out"].astype(np.float64) / den[:, None]).astype(np.float32)

    parts = [None] * NCORES
    if sequential:
        for k, dev in enumerate(devices):
            outs = _dispatch(k, dev)
            res = {name: np.asarray(outs[i]) for i, name in enumerate(out_names)}
            parts[k] = _final(res)
            _log(f"core {k}: done")
    else:
        futures = [_dispatch(k, dev) for k, dev in enumerate(devices)]
        for k, outs in enumerate(futures):
            res = {name: np.asarray(outs[i]) for i, name in enumerate(out_names)}
            parts[k] = _final(res)
            _log(f"core {k}: done")
    return np.concatenate(parts, axis=0)

